# revision 14
# baseline (speedup 1.0000x reference)
"""Trainium2 Bass kernel for GATNet (3-layer GAT+MLP+cdist), 8-core SPMD.

Self-contained: hardcodes shapes/sharding. Inputs are the FULL tensors from
setup_inputs(); output matches reference(): (dist[N,N], (ei[2,E+N], alpha[E+N,2])).

Structure: 3 NEFF phases.
  P1: per-core shard of h_aug = [h | a_s | a_d | 0] (x@W fused with att projections)
  P2: edge aggregation (segment softmax + message matmul) + MLP -> yT, alpha
  P3: 2D-block-sharded cdist
"""

import os
import sys

sys.path.insert(0, "/opt/trn_rl_repo")

import numpy as np

import concourse.bass as bass
import concourse.mybir as mybir
import concourse.tile as tile
from concourse import bacc
from concourse.bass_utils import run_bass_kernel_spmd
from concourse.masks import make_identity

F = mybir.ActivationFunctionType
ALU = mybir.AluOpType

N = 16384
E = 524288
D = 256
HEADS = 2
O = 128
NCORES = 8
NSH = N // NCORES          # 2048 nodes per core
DTILES = NSH // 128        # 16 dst-tiles per core
CH = 6                     # edge-tiles per gather chunk (dma_gather works <=768 idxs)

DT = mybir.dt.float32      # compute dtype knob (float32 | bfloat16)
AUGW = 320                 # h_aug row width in DT elems (1280B f32); %256B
ADW = 64                   # padded-row width for a_d / rdenom gather tables

LAST_EXEC_NS = []          # exec_time_ns per phase when tracing enabled
TRACE = bool(int(os.environ.get("GAT_TRACE", "0")))

_compiled = {}


def _np_of(dt):
    return mybir.dt.np(dt)


def _install_ntff_hook():
    import types

    if "antenv.axon_hooks" in sys.modules:
        return
    mod = types.ModuleType("antenv.axon_hooks")
    mod._hook = None
    mod.set_axon_ntff_profile_hook = lambda h: setattr(mod, "_hook", h)
    mod.get_axon_ntff_profile_hook = lambda: mod._hook
    sys.modules["antenv.axon_hooks"] = mod
    try:
        from trn_agent_boot.trn_boot import _ntff_profile_via_ctypes

        mod.set_axon_ntff_profile_hook(_ntff_profile_via_ctypes("/opt/axon/libaxon_pjrt.so"))
    except Exception:
        pass


def _new_nc():
    return bacc.Bacc("TRN2", target_bir_lowering=False, debug=False, num_devices=NCORES)


# --------------------------------------------------------------------------
# Phase 1: h_aug shard = [x@W (256) | a_s (2) | a_d (2)] per node, cast to DT
# --------------------------------------------------------------------------

def build_p1():
    nc = _new_nc()
    x = nc.dram_tensor("x", [NSH, D], mybir.dt.float32, kind="ExternalInput")
    W = nc.dram_tensor("W", [D, D], mybir.dt.float32, kind="ExternalInput")
    att_src = nc.dram_tensor("att_src", [HEADS, O], mybir.dt.float32, kind="ExternalInput")
    att_dst = nc.dram_tensor("att_dst", [HEADS, O], mybir.dt.float32, kind="ExternalInput")
    haug = nc.dram_tensor("haug", [NSH, AUGW], DT, kind="ExternalOutput")

    with tile.TileContext(nc) as tc:
        with (
            tc.tile_pool(name="const", bufs=1) as cpool,
            tc.tile_pool(name="sbuf", bufs=3) as pool,
            tc.tile_pool(name="psum", bufs=2, space="PSUM") as pp,
            tc.tile_pool(name="psum1", bufs=1, space="PSUM") as pp1,
        ):
            ident = cpool.tile([128, 128], mybir.dt.float32)
            make_identity(nc, ident[:])

            # rhsW[fh] = [W[fh*128:(fh+1)*128, :] (256 cols) | Vsd[fh] (4 cols)]
            rhsW = [cpool.tile([128, D + 4], mybir.dt.float32, name=f"rhsW{i}") for i in range(2)]
            for fh in range(2):
                nc.sync.dma_start(out=rhsW[fh][:, 0:D], in_=W[fh * 128:(fh + 1) * 128, :])

            # attT[h] columns: att_src[h]->col0, att_dst[h]->col1  ([128 o, 2])
            attT = cpool.tile([128, 2 * HEADS], mybir.dt.float32)
            for h in range(HEADS):
                nc.sync.dma_start(out=attT[:, 2 * h:2 * h + 1], in_=att_src[h, :, None])
                nc.sync.dma_start(out=attT[:, 2 * h + 1:2 * h + 2], in_=att_dst[h, :, None])

            # WT quarters: WT[oh][:, f-slice]  (transpose W[fh, oh] blocks)
            for fh in range(2):
                for oh in range(2):
                    wtp = pp.tile([128, 128], mybir.dt.float32, name="wtp")
                    nc.tensor.transpose(out=wtp[:], in_=rhsW[fh][:, oh * 128:(oh + 1) * 128], identity=ident[:])
                    wt = pool.tile([128, 128], mybir.dt.float32, name="wt")
                    nc.vector.tensor_copy(out=wt[:], in_=wtp[:])
                    # Vsd columns for this (fh): head h = oh contract range
                    # haug col layout: 256+ : (s0, s1, d0, d1)
                    vsd = pp1.tile([128, 4], mybir.dt.float32, name="vsd", tag=f"vsd{fh}")
                    h = oh
                    nc.tensor.matmul(vsd[:, h:h + 1], lhsT=wt[:], rhs=attT[:, 2 * h:2 * h + 1],
                                     start=True, stop=True)
                    nc.tensor.matmul(vsd[:, 2 + h:2 + h + 1], lhsT=wt[:], rhs=attT[:, 2 * h + 1:2 * h + 2],
                                     start=True, stop=True)
                    nc.vector.tensor_copy(out=rhsW[fh][:, D + h:D + h + 1], in_=vsd[:, h:h + 1])
                    nc.vector.tensor_copy(out=rhsW[fh][:, D + 2 + h:D + 2 + h + 1], in_=vsd[:, 2 + h:2 + h + 1])

            for t in range(DTILES):
                xt = pool.tile([128, D], mybir.dt.float32, name="xt")
                nc.sync.dma_start(out=xt[:], in_=x[t * 128:(t + 1) * 128, :])
                hp = pp.tile([128, D + 4], mybir.dt.float32, name="hp")
                for fh in range(2):
                    xtp = pp.tile([128, 128], mybir.dt.float32, name="xtp")
                    nc.tensor.transpose(out=xtp[:], in_=xt[:, fh * 128:(fh + 1) * 128], identity=ident[:])
                    xts = pool.tile([128, 128], mybir.dt.float32, name="xts")
                    nc.vector.tensor_copy(out=xts[:], in_=xtp[:])
                    nc.tensor.matmul(hp[:], lhsT=xts[:], rhs=rhsW[fh][:],
                                     start=(fh == 0), stop=(fh == 1))
                ht = pool.tile([128, AUGW], DT, name="ht")
                nc.vector.memset(ht[:, D + 4:AUGW], 0.0)
                nc.vector.tensor_copy(out=ht[:, 0:D + 4], in_=hp[:])
                nc.sync.dma_start(out=haug[t * 128:(t + 1) * 128, :], in_=ht[:])
    nc.compile()
    return nc


# --------------------------------------------------------------------------
# Phase 2: edge aggregation + softmax + MLP
# --------------------------------------------------------------------------

def build_p2(W_T):
    S_T = DTILES * W_T * 128           # padded edge slots per core
    nc = _new_nc()
    f32 = mybir.dt.float32
    haug = nc.dram_tensor("haug", [N + 1, AUGW], DT, kind="ExternalInput")
    adpad = nc.dram_tensor("adpad", [NSH, ADW], DT, kind="ExternalInput")
    srcw = nc.dram_tensor("srcw", [128, S_T // 16], mybir.dt.int16, kind="ExternalInput")
    dstlocw = nc.dram_tensor("dstlocw", [128, S_T // 16], mybir.dt.int16, kind="ExternalInput")
    dstrel_cols = nc.dram_tensor("dstrel_cols", [S_T // 128, 128], DT, kind="ExternalInput")
    b_in = nc.dram_tensor("b", [D], f32, kind="ExternalInput")
    Wa = nc.dram_tensor("Wa", [D, 128], f32, kind="ExternalInput")
    ba = nc.dram_tensor("ba", [128], f32, kind="ExternalInput")
    W1 = nc.dram_tensor("W1", [128, 64], f32, kind="ExternalInput")
    b1 = nc.dram_tensor("b1", [64], f32, kind="ExternalInput")
    W2 = nc.dram_tensor("W2", [64, 3], f32, kind="ExternalInput")
    b2 = nc.dram_tensor("b2", [3], f32, kind="ExternalInput")

    alpha_out = nc.dram_tensor("alpha", [NSH, W_T * 2], f32, kind="ExternalOutput")
    yT_out = nc.dram_tensor("yT", [3, NSH], f32, kind="ExternalOutput")
    rdepad = nc.dram_tensor("rdepad", [NSH, ADW], f32)  # internal

    # chunk layout per dst-tile: W_T edge-tiles split into chunks of <= CH
    chunks = []
    w0 = 0
    while w0 < W_T:
        chunks.append((w0, min(CH, W_T - w0)))
        w0 += chunks[-1][1]

    with tile.TileContext(nc) as tc:
        with (
            tc.tile_pool(name="const", bufs=1) as cpool,
            tc.tile_pool(name="sbuf", bufs=3) as pool,
            tc.tile_pool(name="keep", bufs=len(chunks) + 2) as kpool,
        ):
            ident = cpool.tile([128, 128], mybir.dt.float32)
            make_identity(nc, ident[:])
            iota_i = cpool.tile([128, 128], mybir.dt.int32)
            nc.gpsimd.iota(iota_i[:], pattern=[[1, 128]], base=0, channel_multiplier=0)
            iota_row = cpool.tile([128, 128], DT)
            nc.vector.tensor_copy(out=iota_row[:], in_=iota_i[:])

            h2T = [cpool.tile([128, NSH], f32, name=f"h2T{i}") for i in range(2)]
            b_rep = cpool.tile([128, D], f32)

            agg_psum_ctx = (
                tc.tile_pool(name="psA", bufs=2, space="PSUM"),
                tc.tile_pool(name="psB", bufs=2, space="PSUM"),
                tc.tile_pool(name="psT", bufs=2, space="PSUM"),
            )
            psA, psB, psT = [p.__enter__() for p in agg_psum_ctx]

            # replicate b across partitions via K=1 matmul
            b_row = cpool.tile([1, D], f32)
            nc.sync.dma_start(out=b_row[:], in_=b_in[None, :])
            ones_col = cpool.tile([1, 128], f32)
            nc.vector.memset(ones_col[:], 1.0)
            b_rep_ps = psT.tile([128, D], f32, name="b_rep_ps", bufs=1)
            nc.tensor.matmul(b_rep_ps[:], lhsT=ones_col[:], rhs=b_row[:], start=True, stop=True)
            nc.vector.tensor_copy(out=b_rep[:], in_=b_rep_ps[:])

            for t in range(DTILES):
                agg = psA.tile([128, D], f32, name="agg")
                den = psB.tile([128, 2], f32, name="den")
                wsc_tiles = []
                g1_tiles = []
                for ci, (w0, cw) in enumerate(chunks):
                    s0 = (t * W_T + w0) * 128
                    nidx = cw * 128
                    sw = pool.tile([128, nidx // 16], mybir.dt.int16, name="sw")
                    nc.sync.dma_start(out=sw[:], in_=srcw[:, s0 // 16:(s0 + nidx) // 16])
                    dw = kpool.tile([128, nidx // 16], mybir.dt.int16, name="dw", tag="dw")
                    nc.sync.dma_start(out=dw[:], in_=dstlocw[:, s0 // 16:(s0 + nidx) // 16])
                    drel = pool.tile([128, cw], DT, name="drel")
                    nc.sync.dma_start(out=drel[:], in_=dstrel_cols[s0 // 128:s0 // 128 + cw, :].rearrange("w p -> p w"))

                    g1 = pool.tile([128, CH, AUGW], DT, name="g1", tag="g1")
                    nc.gpsimd.dma_gather(
                        out_ap=g1[:, 0:cw, :], in_ap=haug.ap(), idxs_ap=sw[:],
                        num_idxs=nidx, num_idxs_reg=nidx, elem_size=AUGW,
                    )
                    g1_tiles.append((g1, cw))
                    g2 = pool.tile([128, CH, ADW], DT, name="g2")
                    nc.gpsimd.dma_gather(
                        out_ap=g2[:, 0:cw, :], in_ap=adpad.ap(), idxs_ap=dw[:],
                        num_idxs=nidx, num_idxs_reg=nidx, elem_size=ADW,
                    )
                    # e = leakyrelu(a_s[src] + a_d[dst]); w = exp(e)
                    esc = pool.tile([128, CH, 2], f32, name="esc")
                    nc.vector.tensor_tensor(out=esc[:, 0:cw, :], in0=g1[:, 0:cw, D:D + 2],
                                            in1=g2[:, 0:cw, 0:2], op=ALU.add)
                    nc.scalar.activation(esc[:, 0:cw, :], esc[:, 0:cw, :], F.Prelu,
                                         bias=0.0, scale=1.0, alpha=0.2)
                    wsc = kpool.tile([128, CH, 2], f32, name="wsc", tag="wsc")
                    nc.scalar.activation(wsc[:, 0:cw, :], esc[:, 0:cw, :], F.Exp)
                    wsc_tiles.append((wsc, dw, cw))
                    wdt = wsc
                    if DT != f32:
                        wdt = pool.tile([128, CH, 2], DT, name="wdt")
                        nc.vector.tensor_copy(out=wdt[:, 0:cw, :], in_=wsc[:, 0:cw, :])

                    for w in range(cw):
                        gw = w0 + w
                        S = pool.tile([128, 128], DT, name="S")
                        nc.gpsimd.tensor_scalar(S[:], iota_row[:], drel[:, w:w + 1], None, ALU.is_equal)
                        rhsb = pool.tile([128, 2 * O], DT, name="rhsb")
                        nc.vector.tensor_scalar(rhsb[:, 0:O], g1[:, w, 0:O], wsc[:, w, 0:1], None, ALU.mult)
                        nc.scalar.activation(rhsb[:, O:2 * O], g1[:, w, O:2 * O], F.Copy, bias=0.0,
                                             scale=wsc[:, w, 1:2])
                        first = gw == 0
                        last = gw == W_T - 1
                        nc.tensor.matmul(agg[:], lhsT=S[:], rhs=rhsb[:], start=first, stop=last)
                        nc.tensor.matmul(den[:], lhsT=S[:], rhs=wdt[:, w, 0:2], start=first, stop=last)

                # finalize dst-tile t
                dens = pool.tile([128, 2], f32, name="dens")
                nc.vector.tensor_scalar(dens[:], den[:], 1e-16, None, ALU.add)
                rde = pool.tile([128, 2], f32, name="rde")
                nc.vector.reciprocal(rde[:], dens[:])
                rdew = pool.tile([128, ADW], f32, name="rdew")
                nc.vector.memset(rdew[:, 2:ADW], 0.0)
                nc.vector.tensor_copy(out=rdew[:, 0:2], in_=rde[:])
                nc.sync.dma_start(out=rdepad[t * 128:(t + 1) * 128, :], in_=rdew[:])

                h2t = pool.tile([128, D], f32, name="h2t")
                nc.vector.tensor_scalar(h2t[:, 0:O], agg[:, 0:O], rde[:, 0:1], None, ALU.mult)
                nc.vector.tensor_scalar(h2t[:, O:2 * O], agg[:, O:2 * O], rde[:, 1:2], None, ALU.mult)
                nc.vector.tensor_tensor(out=h2t[:], in0=h2t[:], in1=b_rep[:], op=ALU.add)
                nc.scalar.activation(h2t[:], h2t[:], F.Relu)
                for fh in range(2):
                    htp = psT.tile([128, 128], f32, name="htp")
                    nc.tensor.transpose(out=htp[:], in_=h2t[:, fh * 128:(fh + 1) * 128], identity=ident[:])
                    nc.vector.tensor_copy(out=h2T[fh][:, t * 128:(t + 1) * 128], in_=htp[:])

                # alpha = w * rdenom[dst]
                alpha_t = pool.tile([128, W_T, 2], f32, name="alpha_t")
                for ci, (w0, cw) in enumerate(chunks):
                    wsc, dw, _ = wsc_tiles[ci]
                    g3 = pool.tile([128, CH, ADW], f32, name="g3")
                    nc.gpsimd.dma_gather(
                        out_ap=g3[:, 0:cw, :], in_ap=rdepad.ap(), idxs_ap=dw[:],
                        num_idxs=cw * 128, num_idxs_reg=cw * 128, elem_size=ADW,
                    )
                    nc.vector.tensor_tensor(out=alpha_t[:, w0:w0 + cw, :], in0=wsc[:, 0:cw, :],
                                            in1=g3[:, 0:cw, 0:2], op=ALU.mult)
                nc.sync.dma_start(
                    out=alpha_out[t * 128:(t + 1) * 128, :],
                    in_=alpha_t[:].rearrange("p w c -> p (w c)"),
                )

            for p in reversed(agg_psum_ctx):
                p.__exit__(None, None, None)

            # ---- MLP on h2T -> yT ----
            psM = tc.tile_pool(name="psM", bufs=2, space="PSUM")
            psM_pool = psM.__enter__()
            wa_t = [cpool.tile([128, 128], f32, name=f"wa{i}") for i in range(2)]
            for fh in range(2):
                nc.sync.dma_start(out=wa_t[fh][:], in_=Wa[fh * 128:(fh + 1) * 128, :])
            w1_t = cpool.tile([128, 64], f32)
            nc.sync.dma_start(out=w1_t[:], in_=W1[:, :])
            w2_t = cpool.tile([64, 3], f32)
            nc.sync.dma_start(out=w2_t[:], in_=W2[:, :])
            ba_c = cpool.tile([128, 1], f32)
            nc.sync.dma_start(out=ba_c[:], in_=ba[:, None])
            b1_c = cpool.tile([64, 1], f32)
            nc.sync.dma_start(out=b1_c[:], in_=b1[:, None])
            b2_c = cpool.tile([3, 1], f32)
            nc.sync.dma_start(out=b2_c[:], in_=b2[:, None])

            r1T = cpool.tile([128, NSH], f32)
            r2T = cpool.tile([64, NSH], f32)
            MLPC = min(512, NSH)
            for j in range(NSH // MLPC):
                sl = slice(j * MLPC, (j + 1) * MLPC)
                p1 = psM_pool.tile([128, MLPC], f32, name="p1", tag="mlp1")
                for fh in range(2):
                    nc.tensor.matmul(p1[:], lhsT=wa_t[fh][:], rhs=h2T[fh][:, sl],
                                     start=(fh == 0), stop=(fh == 1))
                nc.scalar.activation(r1T[:, sl], p1[:], F.Relu, bias=ba_c[:])
                p2 = psM_pool.tile([64, MLPC], f32, name="p2", tag="mlp2")
                nc.tensor.matmul(p2[:], lhsT=w1_t[:], rhs=r1T[:, sl], start=True, stop=True)
                nc.scalar.activation(r2T[:, sl], p2[:], F.Relu, bias=b1_c[:])
                p3 = psM_pool.tile([3, MLPC], f32, name="p3", tag="mlp3")
                nc.tensor.matmul(p3[:], lhsT=w2_t[:], rhs=r2T[:, sl], start=True, stop=True)
                yt = pool.tile([3, MLPC], f32, name="yt")
                nc.scalar.activation(yt[:], p3[:], F.Identity, bias=b2_c[:])
                nc.sync.dma_start(out=yT_out[:, sl], in_=yt[:])
            psM.__exit__(None, None, None)
    nc.compile()
    return nc


# --------------------------------------------------------------------------
# Phase 3: dist block = cdist(y_own, y_all)
# --------------------------------------------------------------------------

def build_p3():
    nc = _new_nc()
    f32 = mybir.dt.float32
    yT = nc.dram_tensor("yT", [3, N], f32, kind="ExternalInput")
    yTo = nc.dram_tensor("yTo", [3, NSH], f32, kind="ExternalInput")  # own block
    dist = nc.dram_tensor("dist", [NSH, N], f32, kind="ExternalOutput")

    with tile.TileContext(nc) as tc:
        with (
            tc.tile_pool(name="const", bufs=1) as cpool,
            tc.tile_pool(name="sbuf", bufs=4) as pool,
            tc.tile_pool(name="psum", bufs=4, space="PSUM") as pp,
        ):
            # rhs_cd rows: y (3), ones, y2   [5, N]
            # compute-engine APs must start at partition 0; assemble rows 3,4
            # via SBUF->SBUF DMA from base-0 tiles.
            rhs_cd = cpool.tile([5, N], f32)
            nc.sync.dma_start(out=rhs_cd[0:3, :], in_=yT[:, :])
            CKW = 2048
            ones_c = cpool.tile([1, CKW], f32)
            nc.vector.memset(ones_c[:], 1.0)

            def sumsq_row(src3, dst_slice):  # src3/dst [.., CKW]
                sq = pool.tile([3, CKW], f32, name="sq", tag="sq", bufs=2)
                nc.vector.tensor_tensor(out=sq[:], in0=src3, in1=src3, op=ALU.mult)
                t1 = pool.tile([1, CKW], f32, name="t1", tag="t1", bufs=2)
                nc.sync.dma_start(out=t1[:], in_=sq[1:2, :])
                t2 = pool.tile([1, CKW], f32, name="t2", tag="t2", bufs=2)
                nc.sync.dma_start(out=t2[:], in_=sq[2:3, :])
                y2r = pool.tile([1, CKW], f32, name="y2r", tag="y2r", bufs=2)
                nc.vector.tensor_tensor(out=y2r[:], in0=sq[0:1, :], in1=t1[:], op=ALU.add)
                nc.vector.tensor_tensor(out=y2r[:], in0=y2r[:], in1=t2[:], op=ALU.add)
                nc.sync.dma_start(out=dst_slice, in_=y2r[:])

            for ck in range(N // CKW):
                cs = slice(ck * CKW, (ck + 1) * CKW)
                nc.sync.dma_start(out=rhs_cd[3:4, cs], in_=ones_c[:])
                sumsq_row(rhs_cd[0:3, cs], rhs_cd[4:5, cs])

            # lhsT_cd rows: -2y_own (3), y2_own, ones  [5, NSH]
            lhs_cd = cpool.tile([5, NSH], f32)
            yto_sb = cpool.tile([3, NSH], f32)
            nc.sync.dma_start(out=yto_sb[:], in_=yTo[:, :])
            for ck in range(NSH // CKW):
                cs = slice(ck * CKW, (ck + 1) * CKW)
                nc.sync.dma_start(out=lhs_cd[4:5, cs], in_=ones_c[:])
                sumsq_row(yto_sb[:, cs], lhs_cd[3:4, cs])
            nc.vector.tensor_scalar(lhs_cd[0:3, :], yto_sb[:], -2.0, None, ALU.mult)

            for i in range(DTILES):
                for j in range(N // 512):
                    d2 = pp.tile([128, 512], f32, name="d2")
                    nc.tensor.matmul(d2[:], lhsT=lhs_cd[:, i * 128:(i + 1) * 128],
                                     rhs=rhs_cd[:, j * 512:(j + 1) * 512], start=True, stop=True)
                    dcl = pool.tile([128, 512], f32, name="dcl")
                    nc.vector.tensor_scalar(dcl[:], d2[:], 0.0, None, ALU.max)
                    dst_t = pool.tile([128, 512], f32, name="dst_t")
                    nc.scalar.sqrt(dst_t[:], dcl[:])
                    nc.sync.dma_start(out=dist[i * 128:(i + 1) * 128, j * 512:(j + 1) * 512], in_=dst_t[:])
    nc.compile()
    return nc


# --------------------------------------------------------------------------
# Host orchestration
# --------------------------------------------------------------------------

def _get(phase, key, builder):
    k = (phase, key)
    if k not in _compiled:
        _compiled[k] = builder()
    return _compiled[k]


def _run(nc, in_maps):
    global LAST_EXEC_NS
    if TRACE:
        _install_ntff_hook()
    res = run_bass_kernel_spmd(nc, in_maps, core_ids=list(range(NCORES)), trace=TRACE)
    if TRACE:
        LAST_EXEC_NS.append(res.exec_time_ns)
    return res.results


def host_prep(edge_index):
    """Self loops + sort by dst + pad per dst-tile to uniform W_T*128 slots."""
    ar = np.arange(N, dtype=np.int64)
    ei = np.concatenate([np.asarray(edge_index, np.int64), np.stack([ar, ar])], axis=1)
    Ep = ei.shape[1]
    src_a = ei[0].astype(np.int64)
    dst_a = ei[1].astype(np.int64)
    order = np.argsort(dst_a, kind="stable")
    src_s = src_a[order].astype(np.int32)
    dst_s = dst_a[order].astype(np.int32)
    gtile = dst_s >> 7                                  # global dst-tile
    ntiles = N // 128
    counts = np.bincount(gtile, minlength=ntiles)
    W_T = int(np.ceil(counts.max() / 128))
    S_T = DTILES * W_T * 128
    starts = np.zeros(ntiles, np.int64)
    starts[1:] = np.cumsum(counts)[:-1]
    slot = gtile.astype(np.int64) * (W_T * 128) + (np.arange(Ep) - starts[gtile])

    S_ALL = ntiles * W_T * 128
    src_slots = np.full(S_ALL, N, np.int32)
    src_slots[slot] = src_s
    dstrel_slots = np.zeros(S_ALL, np.int32)
    dstrel_slots[slot] = dst_s & 127
    tile_of_slot = np.arange(S_ALL, dtype=np.int64) // (W_T * 128)
    dstloc_slots = ((tile_of_slot % DTILES) * 128).astype(np.int32)
    dstloc_slots[slot] = dst_s & (NSH - 1)
    return dict(ei=ei, order=order, slot=slot, W_T=W_T, S_T=S_T, S_ALL=S_ALL,
                src_slots=src_slots, dstrel_slots=dstrel_slots,
                dstloc_slots=dstloc_slots)


def wrap16(a):  # [S_T] -> [128, S_T//16] int16
    w = a.reshape(-1, 16).T.astype(np.int16)            # [16, S_T//16]
    return np.tile(w, (8, 1)).copy()


def kernel(x, edge_index, W, b, att_src, att_dst, Wa, ba, W1, b1, W2, b2):
    global LAST_EXEC_NS
    LAST_EXEC_NS = []
    npdt = _np_of(DT)
    x = np.asarray(x, np.float32)
    W = np.asarray(W, np.float32)
    b = np.asarray(b, np.float32)
    att_src = np.asarray(att_src, np.float32)
    att_dst = np.asarray(att_dst, np.float32)
    Wa = np.asarray(Wa, np.float32); ba = np.asarray(ba, np.float32)
    W1 = np.asarray(W1, np.float32); b1 = np.asarray(b1, np.float32)
    W2 = np.asarray(W2, np.float32); b2 = np.asarray(b2, np.float32)
    ei_dt = np.asarray(edge_index).dtype

    hp = host_prep(edge_index)
    ei = hp["ei"]; order = hp["order"]; slot = hp["slot"]
    W_T = hp["W_T"]; S_T = hp["S_T"]; S_ALL = hp["S_ALL"]
    src_slots = hp["src_slots"]; dstrel_slots = hp["dstrel_slots"]
    dstloc_slots = hp["dstloc_slots"]
    Ep = ei.shape[1]

    # ---- phase 1 ----
    nc1 = _get("p1", DT, build_p1)
    p1_in = [
        {"x": x[c * NSH:(c + 1) * NSH], "W": W, "att_src": att_src, "att_dst": att_dst}
        for c in range(NCORES)
    ]
    r1 = _run(nc1, p1_in)
    haug_sh = [r1[c]["haug"] for c in range(NCORES)]
    pad_row = np.zeros((1, AUGW), npdt)
    pad_row[0, D:D + 2] = npdt(-100.0)                   # a_s sentinel -> w=exp(-20)~0
    haug_full = np.concatenate(haug_sh + [pad_row], axis=0)

    # ---- phase 2 ----
    nc2 = _get("p2", (DT, W_T), lambda: build_p2(W_T))
    p2_in = []
    for c in range(NCORES):
        sl = slice(c * S_T, (c + 1) * S_T)
        adp = np.zeros((NSH, ADW), npdt)
        adp[:, 0:2] = haug_full[c * NSH:(c + 1) * NSH, D + 2:D + 4]
        p2_in.append({
            "haug": haug_full,
            "adpad": adp,
            "srcw": wrap16(src_slots[sl]),
            "dstlocw": wrap16(dstloc_slots[sl]),
            "dstrel_cols": dstrel_slots[sl].reshape(-1, 128).astype(npdt),
            "b": b, "Wa": Wa, "ba": ba, "W1": W1, "b1": b1, "W2": W2, "b2": b2,
        })
    r2 = _run(nc2, p2_in)
    yT_full = np.concatenate([r2[c]["yT"] for c in range(NCORES)], axis=1)  # [3, N]

    # alpha: [NSH, W_T*2] per core -> per-slot -> unsort
    alpha_slots = np.empty((S_ALL, 2), np.float32)
    for c in range(NCORES):
        a = r2[c]["alpha"].reshape(NSH, W_T, 2)          # [p-major within tile]
        a = a.reshape(DTILES, 128, W_T, 2).transpose(0, 2, 1, 3).reshape(S_T, 2)
        alpha_slots[c * S_T:(c + 1) * S_T] = a
    alpha_sorted = alpha_slots[slot]                     # [Ep, 2] in sorted order
    alpha = np.empty((Ep, HEADS), np.float32)
    alpha[order] = alpha_sorted

    # ---- phase 3 ----
    nc3 = _get("p3", DT, build_p3)
    p3_in = [{"yT": yT_full, "yTo": np.ascontiguousarray(yT_full[:, c * NSH:(c + 1) * NSH])}
             for c in range(NCORES)]
    r3 = _run(nc3, p3_in)
    dist = np.concatenate([r3[c]["dist"] for c in range(NCORES)], axis=0)

    return dist, (ei.astype(ei_dt), alpha)


# revision 16
# speedup vs baseline: 1.6000x; 1.6000x over previous
"""Trainium2 Bass kernel for GATNet (3-layer GAT+MLP+cdist), 8-core SPMD.

Self-contained: hardcodes shapes/sharding. Inputs are the FULL tensors from
setup_inputs(); output matches reference(): (dist[N,N], (ei[2,E+N], alpha[E+N,2])).

Structure: 3 NEFF phases.
  P1: per-core shard of h_aug = [h | a_s | a_d | 0] (x@W fused with att projections)
  P2: edge aggregation (segment softmax + message matmul) + MLP -> yT, alpha
  P3: 2D-block-sharded cdist
"""

import os
import sys

sys.path.insert(0, "/opt/trn_rl_repo")

import numpy as np

import concourse.bass as bass
import concourse.mybir as mybir
import concourse.tile as tile
from concourse import bacc
from concourse.bass_utils import run_bass_kernel_spmd
from concourse.masks import make_identity

F = mybir.ActivationFunctionType
ALU = mybir.AluOpType

N = 16384
E = 524288
D = 256
HEADS = 2
O = 128
NCORES = 8
NSH = N // NCORES          # 2048 nodes per core
DTILES = NSH // 128        # 16 dst-tiles per core
CH = 6                     # edge-tiles per gather chunk (dma_gather works <=768 idxs)

DT = mybir.dt.float32      # compute dtype knob (float32 | bfloat16)
AUGW = 320                 # h_aug row width in DT elems (1280B f32); %256B
ADW = 64                   # padded-row width for a_d / rdenom gather tables

LAST_EXEC_NS = []          # exec_time_ns per phase when tracing enabled
TRACE = bool(int(os.environ.get("GAT_TRACE", "0")))

_compiled = {}


def _np_of(dt):
    return mybir.dt.np(dt)


def _install_ntff_hook():
    import types

    if "antenv.axon_hooks" in sys.modules:
        return
    mod = types.ModuleType("antenv.axon_hooks")
    mod._hook = None
    mod.set_axon_ntff_profile_hook = lambda h: setattr(mod, "_hook", h)
    mod.get_axon_ntff_profile_hook = lambda: mod._hook
    sys.modules["antenv.axon_hooks"] = mod
    try:
        from trn_agent_boot.trn_boot import _ntff_profile_via_ctypes

        mod.set_axon_ntff_profile_hook(_ntff_profile_via_ctypes("/opt/axon/libaxon_pjrt.so"))
    except Exception:
        pass


def _new_nc():
    return bacc.Bacc("TRN2", target_bir_lowering=False, debug=False, num_devices=NCORES)


# --------------------------------------------------------------------------
# Phase 1: h_aug shard = [x@W (256) | a_s (2) | a_d (2)] per node, cast to DT
# --------------------------------------------------------------------------

def build_p1():
    nc = _new_nc()
    x = nc.dram_tensor("x", [NSH, D], mybir.dt.float32, kind="ExternalInput")
    W = nc.dram_tensor("W", [D, D], mybir.dt.float32, kind="ExternalInput")
    att_src = nc.dram_tensor("att_src", [HEADS, O], mybir.dt.float32, kind="ExternalInput")
    att_dst = nc.dram_tensor("att_dst", [HEADS, O], mybir.dt.float32, kind="ExternalInput")
    haug = nc.dram_tensor("haug", [NSH, AUGW], DT, kind="ExternalOutput")

    with tile.TileContext(nc) as tc:
        with (
            tc.tile_pool(name="const", bufs=1) as cpool,
            tc.tile_pool(name="sbuf", bufs=3) as pool,
            tc.tile_pool(name="psum", bufs=2, space="PSUM") as pp,
            tc.tile_pool(name="psum1", bufs=1, space="PSUM") as pp1,
        ):
            ident = cpool.tile([128, 128], mybir.dt.float32)
            make_identity(nc, ident[:])

            # rhsW[fh] = [W[fh*128:(fh+1)*128, :] (256 cols) | Vsd[fh] (4 cols)]
            rhsW = [cpool.tile([128, D + 4], mybir.dt.float32, name=f"rhsW{i}") for i in range(2)]
            for fh in range(2):
                nc.sync.dma_start(out=rhsW[fh][:, 0:D], in_=W[fh * 128:(fh + 1) * 128, :])

            # attT[h] columns: att_src[h]->col0, att_dst[h]->col1  ([128 o, 2])
            attT = cpool.tile([128, 2 * HEADS], mybir.dt.float32)
            for h in range(HEADS):
                nc.sync.dma_start(out=attT[:, 2 * h:2 * h + 1], in_=att_src[h, :, None])
                nc.sync.dma_start(out=attT[:, 2 * h + 1:2 * h + 2], in_=att_dst[h, :, None])

            # WT quarters: WT[oh][:, f-slice]  (transpose W[fh, oh] blocks)
            for fh in range(2):
                for oh in range(2):
                    wtp = pp.tile([128, 128], mybir.dt.float32, name="wtp")
                    nc.tensor.transpose(out=wtp[:], in_=rhsW[fh][:, oh * 128:(oh + 1) * 128], identity=ident[:])
                    wt = pool.tile([128, 128], mybir.dt.float32, name="wt")
                    nc.vector.tensor_copy(out=wt[:], in_=wtp[:])
                    # Vsd columns for this (fh): head h = oh contract range
                    # haug col layout: 256+ : (s0, s1, d0, d1)
                    vsd = pp1.tile([128, 4], mybir.dt.float32, name="vsd", tag=f"vsd{fh}")
                    h = oh
                    nc.tensor.matmul(vsd[:, h:h + 1], lhsT=wt[:], rhs=attT[:, 2 * h:2 * h + 1],
                                     start=True, stop=True)
                    nc.tensor.matmul(vsd[:, 2 + h:2 + h + 1], lhsT=wt[:], rhs=attT[:, 2 * h + 1:2 * h + 2],
                                     start=True, stop=True)
                    nc.vector.tensor_copy(out=rhsW[fh][:, D + h:D + h + 1], in_=vsd[:, h:h + 1])
                    nc.vector.tensor_copy(out=rhsW[fh][:, D + 2 + h:D + 2 + h + 1], in_=vsd[:, 2 + h:2 + h + 1])

            for t in range(DTILES):
                xt = pool.tile([128, D], mybir.dt.float32, name="xt")
                nc.sync.dma_start(out=xt[:], in_=x[t * 128:(t + 1) * 128, :])
                hp = pp.tile([128, D + 4], mybir.dt.float32, name="hp")
                for fh in range(2):
                    xtp = pp.tile([128, 128], mybir.dt.float32, name="xtp")
                    nc.tensor.transpose(out=xtp[:], in_=xt[:, fh * 128:(fh + 1) * 128], identity=ident[:])
                    xts = pool.tile([128, 128], mybir.dt.float32, name="xts")
                    nc.vector.tensor_copy(out=xts[:], in_=xtp[:])
                    nc.tensor.matmul(hp[:], lhsT=xts[:], rhs=rhsW[fh][:],
                                     start=(fh == 0), stop=(fh == 1))
                ht = pool.tile([128, AUGW], DT, name="ht")
                nc.vector.memset(ht[:, D + 4:AUGW], 0.0)
                nc.vector.tensor_copy(out=ht[:, 0:D + 4], in_=hp[:])
                nc.sync.dma_start(out=haug[t * 128:(t + 1) * 128, :], in_=ht[:])
    nc.compile()
    return nc


# --------------------------------------------------------------------------
# Phase 2: edge aggregation + softmax + MLP
# --------------------------------------------------------------------------

def build_p2(W_T):
    S_T = DTILES * W_T * 128           # padded edge slots per core
    nc = _new_nc()
    f32 = mybir.dt.float32
    haug = nc.dram_tensor("haug", [N + 1, AUGW], DT, kind="ExternalInput")
    srcw = nc.dram_tensor("srcw", [128, S_T // 16], mybir.dt.int16, kind="ExternalInput")
    dstrel_cols = nc.dram_tensor("dstrel_cols", [S_T // 128, 128], DT, kind="ExternalInput")
    dstrel_flat = nc.dram_tensor("dstrel_flat", [1, S_T], DT, kind="ExternalInput")
    asd = nc.dram_tensor("asd", [NSH, 2], DT, kind="ExternalInput")
    b_rep_in = nc.dram_tensor("b_rep", [128, D], f32, kind="ExternalInput")
    Wa = nc.dram_tensor("Wa", [D, 128], f32, kind="ExternalInput")
    ba = nc.dram_tensor("ba", [128], f32, kind="ExternalInput")
    W1 = nc.dram_tensor("W1", [128, 64], f32, kind="ExternalInput")
    b1 = nc.dram_tensor("b1", [64], f32, kind="ExternalInput")
    W2 = nc.dram_tensor("W2", [64, 3], f32, kind="ExternalInput")
    b2 = nc.dram_tensor("b2", [3], f32, kind="ExternalInput")

    alpha_out = nc.dram_tensor("alpha", [NSH, W_T * 2], f32, kind="ExternalOutput")
    yT_out = nc.dram_tensor("yT", [3, NSH], f32, kind="ExternalOutput")

    # chunk layout per dst-tile: W_T edge-tiles split into chunks of <= CH
    chunks = []
    w0 = 0
    while w0 < W_T:
        chunks.append((w0, min(CH, W_T - w0)))
        w0 += chunks[-1][1]

    with tile.TileContext(nc) as tc:
        with (
            tc.tile_pool(name="const", bufs=1) as cpool,
            tc.tile_pool(name="sbuf", bufs=3) as pool,
            tc.tile_pool(name="keep", bufs=len(chunks) + 2) as kpool,
        ):
            ident = cpool.tile([128, 128], mybir.dt.float32)
            make_identity(nc, ident[:])
            iota_i = cpool.tile([128, 128], mybir.dt.int32)
            nc.gpsimd.iota(iota_i[:], pattern=[[1, 128]], base=0, channel_multiplier=0)
            iota_row = cpool.tile([128, 128], DT)
            nc.vector.tensor_copy(out=iota_row[:], in_=iota_i[:])
            iotac_i = cpool.tile([128, 1], mybir.dt.int32)
            nc.gpsimd.iota(iotac_i[:], pattern=[[1, 1]], base=0, channel_multiplier=1)
            iota_col = cpool.tile([128, 1], DT)
            nc.vector.tensor_copy(out=iota_col[:], in_=iotac_i[:])

            b_rep = cpool.tile([128, D], f32)
            nc.sync.dma_start(out=b_rep[:], in_=b_rep_in[:, :])
            h2T = [cpool.tile([128, NSH], f32, name=f"h2T{i}") for i in range(2)]

            agg_psum_ctx = (
                tc.tile_pool(name="psA", bufs=1, space="PSUM"),
                tc.tile_pool(name="psB", bufs=1, space="PSUM"),
            )
            psA, psB = [p.__enter__() for p in agg_psum_ctx]

            for t in range(DTILES):
                agg = psA.tile([128, D], f32, name="agg", tag="agg", bufs=2)
                den = psB.tile([128, 2], f32, name="den", tag="den", bufs=1)
                asd_t = pool.tile([128, 2], DT, name="asd_t")
                nc.sync.dma_start(out=asd_t[:], in_=asd[t * 128:(t + 1) * 128, :])
                wsc_tiles = []
                s2_tiles = []
                for ci, (w0, cw) in enumerate(chunks):
                    s0 = (t * W_T + w0) * 128
                    nidx = cw * 128
                    sw = pool.tile([128, nidx // 16], mybir.dt.int16, name="sw")
                    nc.sync.dma_start(out=sw[:], in_=srcw[:, s0 // 16:(s0 + nidx) // 16])
                    drel = pool.tile([128, cw], DT, name="drel")
                    nc.sync.dma_start(out=drel[:], in_=dstrel_cols[s0 // 128:s0 // 128 + cw, :].rearrange("w p -> p w"))
                    drep = pool.tile([128, CH * 128], DT, name="drep")
                    nc.sync.dma_start(out=drep[:, 0:nidx],
                                      in_=dstrel_flat[0:1, s0:s0 + nidx].to_broadcast([128, nidx]))

                    g1 = pool.tile([128, CH, AUGW], DT, name="g1", tag="g1")
                    nc.gpsimd.dma_gather(
                        out_ap=g1[:, 0:cw, :], in_ap=haug.ap(), idxs_ap=sw[:],
                        num_idxs=nidx, num_idxs_reg=nidx, elem_size=AUGW,
                    )
                    # S2[d, e] one-hot + a_d expansion via matmul
                    ade = psB.tile([128, 2 * CH], f32, name="ade", tag="ade", bufs=2)
                    cw_s2 = []
                    for w in range(cw):
                        S2 = kpool.tile([128, 128], DT, name="S2", tag="S2", bufs=W_T + 4)
                        nc.vector.tensor_scalar(S2[:], drep[:, w * 128:(w + 1) * 128],
                                                iota_col[:], None, ALU.is_equal)
                        nc.tensor.matmul(ade[:, 2 * w:2 * w + 2], lhsT=S2[:], rhs=asd_t[:],
                                         start=True, stop=True)
                        cw_s2.append(S2)
                    s2_tiles.append(cw_s2)
                    # e = leakyrelu(a_s[src] + a_d[dst]) = max(e0, 0.2*e0); w = exp(e)
                    esc = pool.tile([128, CH, 2], f32, name="esc")
                    nc.vector.tensor_tensor(out=esc[:, 0:cw, :], in0=g1[:, 0:cw, D:D + 2],
                                            in1=ade[:, 0:2 * cw].rearrange("p (w c) -> p w c", c=2),
                                            op=ALU.add)
                    esc2 = pool.tile([128, CH, 2], f32, name="esc2")
                    nc.vector.tensor_scalar(esc2[:, 0:cw, :], esc[:, 0:cw, :], 0.2, None, ALU.mult)
                    nc.vector.tensor_tensor(out=esc[:, 0:cw, :], in0=esc[:, 0:cw, :],
                                            in1=esc2[:, 0:cw, :], op=ALU.max)
                    wsc = kpool.tile([128, CH, 2], f32, name="wsc", tag="wsc")
                    nc.scalar.activation(wsc[:, 0:cw, :], esc[:, 0:cw, :], F.Exp)
                    wsc_tiles.append(wsc)
                    wbf = pool.tile([128, CH, 2], DT, name="wbf")
                    nc.vector.tensor_copy(out=wbf[:, 0:cw, :], in_=wsc[:, 0:cw, :])

                    for w in range(cw):
                        gw = w0 + w
                        S = pool.tile([128, 128], DT, name="S")
                        nc.vector.tensor_scalar(S[:], iota_row[:], drel[:, w:w + 1], None, ALU.is_equal)
                        rhsb = pool.tile([128, 2 * O], DT, name="rhsb")
                        nc.vector.tensor_scalar(rhsb[:, 0:O], g1[:, w, 0:O], wbf[:, w, 0:1], None, ALU.mult)
                        nc.vector.tensor_scalar(rhsb[:, O:2 * O], g1[:, w, O:2 * O], wbf[:, w, 1:2], None, ALU.mult)
                        first = gw == 0
                        last = gw == W_T - 1
                        nc.tensor.matmul(agg[:], lhsT=S[:], rhs=rhsb[:], start=first, stop=last)
                        nc.tensor.matmul(den[:], lhsT=S[:], rhs=wbf[:, w, 0:2], start=first, stop=last)

                # finalize dst-tile t
                dens = pool.tile([128, 2], f32, name="dens")
                nc.vector.tensor_scalar(dens[:], den[:], 1e-16, None, ALU.add)
                rde = pool.tile([128, 2], f32, name="rde")
                nc.vector.reciprocal(rde[:], dens[:])
                rde_dt = pool.tile([128, 2], DT, name="rde_dt")
                nc.vector.tensor_copy(out=rde_dt[:], in_=rde[:])

                h2t = pool.tile([128, D], f32, name="h2t")
                nc.vector.tensor_scalar(h2t[:, 0:O], agg[:, 0:O], rde[:, 0:1], None, ALU.mult)
                nc.vector.tensor_scalar(h2t[:, O:2 * O], agg[:, O:2 * O], rde[:, 1:2], None, ALU.mult)
                nc.vector.tensor_tensor(out=h2t[:], in0=h2t[:], in1=b_rep[:], op=ALU.add)
                nc.vector.tensor_scalar(h2t[:], h2t[:], 0.0, None, ALU.max)
                for fh in range(2):
                    htp = psA.tile([128, 128], f32, name="htp", tag="htp", bufs=1)
                    nc.tensor.transpose(out=htp[:], in_=h2t[:, fh * 128:(fh + 1) * 128], identity=ident[:])
                    nc.vector.tensor_copy(out=h2T[fh][:, t * 128:(t + 1) * 128], in_=htp[:])

                # alpha = w * rdenom[dst] (expand rdenom via kept S2 tiles)
                alpha_t = pool.tile([128, W_T, 2], f32, name="alpha_t")
                for ci, (w0, cw) in enumerate(chunks):
                    wsc = wsc_tiles[ci]
                    rdexp = psB.tile([128, 2 * CH], f32, name="rdexp", tag="rdexp", bufs=1)
                    for w in range(cw):
                        nc.tensor.matmul(rdexp[:, 2 * w:2 * w + 2], lhsT=s2_tiles[ci][w][:],
                                         rhs=rde_dt[:], start=True, stop=True)
                    nc.vector.tensor_tensor(out=alpha_t[:, w0:w0 + cw, :], in0=wsc[:, 0:cw, :],
                                            in1=rdexp[:, 0:2 * cw].rearrange("p (w c) -> p w c", c=2),
                                            op=ALU.mult)
                nc.sync.dma_start(
                    out=alpha_out[t * 128:(t + 1) * 128, :],
                    in_=alpha_t[:].rearrange("p w c -> p (w c)"),
                )

            for p in reversed(agg_psum_ctx):
                p.__exit__(None, None, None)

            # ---- MLP on h2T -> yT (DVE for bias+relu; no ACT tables) ----
            psM = tc.tile_pool(name="psM", bufs=2, space="PSUM")
            psM_pool = psM.__enter__()
            wa_t = [cpool.tile([128, 128], f32, name=f"wa{i}") for i in range(2)]
            for fh in range(2):
                nc.sync.dma_start(out=wa_t[fh][:], in_=Wa[fh * 128:(fh + 1) * 128, :])
            w1_t = cpool.tile([128, 64], f32)
            nc.sync.dma_start(out=w1_t[:], in_=W1[:, :])
            w2_t = cpool.tile([64, 3], f32)
            nc.sync.dma_start(out=w2_t[:], in_=W2[:, :])
            ba_c = cpool.tile([128, 1], f32)
            nc.sync.dma_start(out=ba_c[:], in_=ba[:, None])
            b1_c = cpool.tile([64, 1], f32)
            nc.sync.dma_start(out=b1_c[:], in_=b1[:, None])
            b2_c = cpool.tile([3, 1], f32)
            nc.sync.dma_start(out=b2_c[:], in_=b2[:, None])

            r1T = cpool.tile([128, NSH], f32)
            r2T = cpool.tile([64, NSH], f32)
            MLPC = min(512, NSH)
            for j in range(NSH // MLPC):
                sl = slice(j * MLPC, (j + 1) * MLPC)
                p1 = psM_pool.tile([128, MLPC], f32, name="p1", tag="mlp1")
                for fh in range(2):
                    nc.tensor.matmul(p1[:], lhsT=wa_t[fh][:], rhs=h2T[fh][:, sl],
                                     start=(fh == 0), stop=(fh == 1))
                nc.vector.tensor_scalar(r1T[:, sl], p1[:], ba_c[:], 0.0, ALU.add, ALU.max)
                p2 = psM_pool.tile([64, MLPC], f32, name="p2", tag="mlp2")
                nc.tensor.matmul(p2[:], lhsT=w1_t[:], rhs=r1T[:, sl], start=True, stop=True)
                nc.vector.tensor_scalar(r2T[:, sl], p2[:], b1_c[:], 0.0, ALU.add, ALU.max)
                p3 = psM_pool.tile([3, MLPC], f32, name="p3", tag="mlp3")
                nc.tensor.matmul(p3[:], lhsT=w2_t[:], rhs=r2T[:, sl], start=True, stop=True)
                yt = pool.tile([3, MLPC], f32, name="yt")
                nc.vector.tensor_scalar(yt[:], p3[:], b2_c[:], None, ALU.add)
                nc.sync.dma_start(out=yT_out[:, sl], in_=yt[:])
            psM.__exit__(None, None, None)
    nc.compile()
    return nc


# --------------------------------------------------------------------------
# Phase 3: dist block = cdist(y_own, y_all)
# --------------------------------------------------------------------------

def build_p3():
    nc = _new_nc()
    f32 = mybir.dt.float32
    yT = nc.dram_tensor("yT", [3, N], f32, kind="ExternalInput")
    yTo = nc.dram_tensor("yTo", [3, NSH], f32, kind="ExternalInput")  # own block
    dist = nc.dram_tensor("dist", [NSH, N], f32, kind="ExternalOutput")

    with tile.TileContext(nc) as tc:
        with (
            tc.tile_pool(name="const", bufs=1) as cpool,
            tc.tile_pool(name="sbuf", bufs=4) as pool,
            tc.tile_pool(name="psum", bufs=4, space="PSUM") as pp,
        ):
            # rhs_cd rows: y (3), ones, y2   [5, N]
            # compute-engine APs must start at partition 0; assemble rows 3,4
            # via SBUF->SBUF DMA from base-0 tiles.
            rhs_cd = cpool.tile([5, N], f32)
            nc.sync.dma_start(out=rhs_cd[0:3, :], in_=yT[:, :])
            CKW = 2048
            ones_c = cpool.tile([1, CKW], f32)
            nc.vector.memset(ones_c[:], 1.0)

            def sumsq_row(src3, dst_slice):  # src3/dst [.., CKW]
                sq = pool.tile([3, CKW], f32, name="sq", tag="sq", bufs=2)
                nc.vector.tensor_tensor(out=sq[:], in0=src3, in1=src3, op=ALU.mult)
                t1 = pool.tile([1, CKW], f32, name="t1", tag="t1", bufs=2)
                nc.sync.dma_start(out=t1[:], in_=sq[1:2, :])
                t2 = pool.tile([1, CKW], f32, name="t2", tag="t2", bufs=2)
                nc.sync.dma_start(out=t2[:], in_=sq[2:3, :])
                y2r = pool.tile([1, CKW], f32, name="y2r", tag="y2r", bufs=2)
                nc.vector.tensor_tensor(out=y2r[:], in0=sq[0:1, :], in1=t1[:], op=ALU.add)
                nc.vector.tensor_tensor(out=y2r[:], in0=y2r[:], in1=t2[:], op=ALU.add)
                nc.sync.dma_start(out=dst_slice, in_=y2r[:])

            for ck in range(N // CKW):
                cs = slice(ck * CKW, (ck + 1) * CKW)
                nc.sync.dma_start(out=rhs_cd[3:4, cs], in_=ones_c[:])
                sumsq_row(rhs_cd[0:3, cs], rhs_cd[4:5, cs])

            # lhsT_cd rows: -2y_own (3), y2_own, ones  [5, NSH]
            lhs_cd = cpool.tile([5, NSH], f32)
            yto_sb = cpool.tile([3, NSH], f32)
            nc.sync.dma_start(out=yto_sb[:], in_=yTo[:, :])
            for ck in range(NSH // CKW):
                cs = slice(ck * CKW, (ck + 1) * CKW)
                nc.sync.dma_start(out=lhs_cd[4:5, cs], in_=ones_c[:])
                sumsq_row(yto_sb[:, cs], lhs_cd[3:4, cs])
            nc.vector.tensor_scalar(lhs_cd[0:3, :], yto_sb[:], -2.0, None, ALU.mult)

            for i in range(DTILES):
                for j in range(N // 512):
                    d2 = pp.tile([128, 512], f32, name="d2")
                    nc.tensor.matmul(d2[:], lhsT=lhs_cd[:, i * 128:(i + 1) * 128],
                                     rhs=rhs_cd[:, j * 512:(j + 1) * 512], start=True, stop=True)
                    dcl = pool.tile([128, 512], f32, name="dcl")
                    nc.vector.tensor_scalar(dcl[:], d2[:], 0.0, None, ALU.max)
                    dst_t = pool.tile([128, 512], f32, name="dst_t")
                    nc.scalar.sqrt(dst_t[:], dcl[:])
                    nc.sync.dma_start(out=dist[i * 128:(i + 1) * 128, j * 512:(j + 1) * 512], in_=dst_t[:])
    nc.compile()
    return nc


# --------------------------------------------------------------------------
# Host orchestration
# --------------------------------------------------------------------------

def _get(phase, key, builder):
    k = (phase, key)
    if k not in _compiled:
        _compiled[k] = builder()
    return _compiled[k]


def _run(nc, in_maps):
    global LAST_EXEC_NS
    if TRACE:
        _install_ntff_hook()
    res = run_bass_kernel_spmd(nc, in_maps, core_ids=list(range(NCORES)), trace=TRACE)
    if TRACE:
        LAST_EXEC_NS.append(res.exec_time_ns)
    return res.results


def host_prep(edge_index):
    """Self loops + sort by dst + pad per dst-tile to uniform W_T*128 slots."""
    ar = np.arange(N, dtype=np.int64)
    ei = np.concatenate([np.asarray(edge_index, np.int64), np.stack([ar, ar])], axis=1)
    Ep = ei.shape[1]
    src_a = ei[0].astype(np.int64)
    dst_a = ei[1].astype(np.int64)
    order = np.argsort(dst_a, kind="stable")
    src_s = src_a[order].astype(np.int32)
    dst_s = dst_a[order].astype(np.int32)
    gtile = dst_s >> 7                                  # global dst-tile
    ntiles = N // 128
    counts = np.bincount(gtile, minlength=ntiles)
    W_T = int(np.ceil(counts.max() / 128))
    S_T = DTILES * W_T * 128
    starts = np.zeros(ntiles, np.int64)
    starts[1:] = np.cumsum(counts)[:-1]
    slot = gtile.astype(np.int64) * (W_T * 128) + (np.arange(Ep) - starts[gtile])

    S_ALL = ntiles * W_T * 128
    src_slots = np.full(S_ALL, N, np.int32)
    src_slots[slot] = src_s
    dstrel_slots = np.zeros(S_ALL, np.int32)
    dstrel_slots[slot] = dst_s & 127
    tile_of_slot = np.arange(S_ALL, dtype=np.int64) // (W_T * 128)
    dstloc_slots = ((tile_of_slot % DTILES) * 128).astype(np.int32)
    dstloc_slots[slot] = dst_s & (NSH - 1)
    return dict(ei=ei, order=order, slot=slot, W_T=W_T, S_T=S_T, S_ALL=S_ALL,
                src_slots=src_slots, dstrel_slots=dstrel_slots,
                dstloc_slots=dstloc_slots)


def wrap16(a):  # [S_T] -> [128, S_T//16] int16
    w = a.reshape(-1, 16).T.astype(np.int16)            # [16, S_T//16]
    return np.tile(w, (8, 1)).copy()


def kernel(x, edge_index, W, b, att_src, att_dst, Wa, ba, W1, b1, W2, b2):
    global LAST_EXEC_NS
    LAST_EXEC_NS = []
    npdt = _np_of(DT)
    x = np.asarray(x, np.float32)
    W = np.asarray(W, np.float32)
    b = np.asarray(b, np.float32)
    att_src = np.asarray(att_src, np.float32)
    att_dst = np.asarray(att_dst, np.float32)
    Wa = np.asarray(Wa, np.float32); ba = np.asarray(ba, np.float32)
    W1 = np.asarray(W1, np.float32); b1 = np.asarray(b1, np.float32)
    W2 = np.asarray(W2, np.float32); b2 = np.asarray(b2, np.float32)
    ei_dt = np.asarray(edge_index).dtype

    hp = host_prep(edge_index)
    ei = hp["ei"]; order = hp["order"]; slot = hp["slot"]
    W_T = hp["W_T"]; S_T = hp["S_T"]; S_ALL = hp["S_ALL"]
    src_slots = hp["src_slots"]; dstrel_slots = hp["dstrel_slots"]
    dstloc_slots = hp["dstloc_slots"]
    Ep = ei.shape[1]

    # ---- phase 1 ----
    nc1 = _get("p1", DT, build_p1)
    p1_in = [
        {"x": x[c * NSH:(c + 1) * NSH], "W": W, "att_src": att_src, "att_dst": att_dst}
        for c in range(NCORES)
    ]
    r1 = _run(nc1, p1_in)
    haug_sh = [r1[c]["haug"] for c in range(NCORES)]
    pad_row = np.zeros((1, AUGW), npdt)
    pad_row[0, D:D + 2] = npdt(-100.0)                   # a_s sentinel -> w=exp(-20)~0
    haug_full = np.concatenate(haug_sh + [pad_row], axis=0)

    # ---- phase 2 ----
    nc2 = _get("p2", (DT, W_T), lambda: build_p2(W_T))
    b_rep = np.tile(b[None, :], (128, 1)).astype(np.float32)
    p2_in = []
    for c in range(NCORES):
        sl = slice(c * S_T, (c + 1) * S_T)
        drel = dstrel_slots[sl].astype(npdt)
        p2_in.append({
            "haug": haug_full,
            "srcw": wrap16(src_slots[sl]),
            "dstrel_cols": drel.reshape(-1, 128),
            "dstrel_flat": drel.reshape(1, -1),
            "asd": np.ascontiguousarray(haug_full[c * NSH:(c + 1) * NSH, D + 2:D + 4]),
            "b_rep": b_rep, "Wa": Wa, "ba": ba, "W1": W1, "b1": b1, "W2": W2, "b2": b2,
        })
    r2 = _run(nc2, p2_in)
    yT_full = np.concatenate([r2[c]["yT"] for c in range(NCORES)], axis=1)  # [3, N]

    # alpha: [NSH, W_T*2] per core -> per-slot -> unsort
    alpha_slots = np.empty((S_ALL, 2), np.float32)
    for c in range(NCORES):
        a = r2[c]["alpha"].reshape(NSH, W_T, 2)          # [p-major within tile]
        a = a.reshape(DTILES, 128, W_T, 2).transpose(0, 2, 1, 3).reshape(S_T, 2)
        alpha_slots[c * S_T:(c + 1) * S_T] = a
    alpha_sorted = alpha_slots[slot]                     # [Ep, 2] in sorted order
    alpha = np.empty((Ep, HEADS), np.float32)
    alpha[order] = alpha_sorted

    # ---- phase 3 ----
    nc3 = _get("p3", DT, build_p3)
    p3_in = [{"yT": yT_full, "yTo": np.ascontiguousarray(yT_full[:, c * NSH:(c + 1) * NSH])}
             for c in range(NCORES)]
    r3 = _run(nc3, p3_in)
    dist = np.concatenate([r3[c]["dist"] for c in range(NCORES)], axis=0)

    return dist, (ei.astype(ei_dt), alpha)


# revision 20
# speedup vs baseline: 2.0848x; 1.3030x over previous
"""Trainium2 Bass kernel for GATNet (3-layer GAT+MLP+cdist), 8-core SPMD.

Self-contained: hardcodes shapes/sharding. Inputs are the FULL tensors from
setup_inputs(); output matches reference(): (dist[N,N], (ei[2,E+N], alpha[E+N,2])).

Structure: 3 NEFF phases.
  P1: per-core shard of h_aug = [h | a_s | a_d | 0] (x@W fused with att projections)
  P2: edge aggregation (segment softmax + message matmul) + MLP -> yT, alpha
  P3: 2D-block-sharded cdist
"""

import os
import sys

sys.path.insert(0, "/opt/trn_rl_repo")

import numpy as np

import concourse.bass as bass
import concourse.mybir as mybir
import concourse.tile as tile
from concourse import bacc
from concourse.bass_utils import run_bass_kernel_spmd
from concourse.masks import make_identity

F = mybir.ActivationFunctionType
ALU = mybir.AluOpType

N = 16384
E = 524288
D = 256
HEADS = 2
O = 128
NCORES = 8
NSH = N // NCORES          # 2048 nodes per core
DTILES = NSH // 128        # 16 dst-tiles per core
CH = 6                     # edge-tiles per gather chunk (dma_gather works <=768 idxs)

DT = mybir.dt.bfloat16     # compute dtype knob (float32 | bfloat16)
# h_aug row width in DT elems; row bytes must be a multiple of 256
AUGW = 320 if DT == mybir.dt.float32 else 384

LAST_EXEC_NS = []          # exec_time_ns per phase when tracing enabled
TRACE = bool(int(os.environ.get("GAT_TRACE", "0")))

_compiled = {}


def _np_of(dt):
    return mybir.dt.np(dt)


def _install_ntff_hook():
    import types

    if "antenv.axon_hooks" in sys.modules:
        return
    mod = types.ModuleType("antenv.axon_hooks")
    mod._hook = None
    mod.set_axon_ntff_profile_hook = lambda h: setattr(mod, "_hook", h)
    mod.get_axon_ntff_profile_hook = lambda: mod._hook
    sys.modules["antenv.axon_hooks"] = mod
    try:
        from trn_agent_boot.trn_boot import _ntff_profile_via_ctypes

        mod.set_axon_ntff_profile_hook(_ntff_profile_via_ctypes("/opt/axon/libaxon_pjrt.so"))
    except Exception:
        pass


def _new_nc():
    return bacc.Bacc("TRN2", target_bir_lowering=False, debug=False, num_devices=NCORES)


# --------------------------------------------------------------------------
# Phase 1: h_aug shard = [x@W (256) | a_s (2) | a_d (2)] per node, cast to DT
# --------------------------------------------------------------------------

def build_p1():
    nc = _new_nc()
    x = nc.dram_tensor("x", [NSH, D], mybir.dt.float32, kind="ExternalInput")
    W = nc.dram_tensor("W", [D, D], mybir.dt.float32, kind="ExternalInput")
    att_src = nc.dram_tensor("att_src", [HEADS, O], mybir.dt.float32, kind="ExternalInput")
    att_dst = nc.dram_tensor("att_dst", [HEADS, O], mybir.dt.float32, kind="ExternalInput")
    haug = nc.dram_tensor("haug", [NSH, AUGW], DT, kind="ExternalOutput")

    with tile.TileContext(nc) as tc:
        with (
            tc.tile_pool(name="const", bufs=1) as cpool,
            tc.tile_pool(name="sbuf", bufs=3) as pool,
            tc.tile_pool(name="psum", bufs=2, space="PSUM") as pp,
            tc.tile_pool(name="psum1", bufs=1, space="PSUM") as pp1,
        ):
            ident = cpool.tile([128, 128], mybir.dt.float32)
            make_identity(nc, ident[:])

            # rhsW[fh] = [W[fh*128:(fh+1)*128, :] (256 cols) | Vsd[fh] (4 cols)]
            rhsW = [cpool.tile([128, D + 4], mybir.dt.float32, name=f"rhsW{i}") for i in range(2)]
            for fh in range(2):
                nc.sync.dma_start(out=rhsW[fh][:, 0:D], in_=W[fh * 128:(fh + 1) * 128, :])

            # attT[h] columns: att_src[h]->col0, att_dst[h]->col1  ([128 o, 2])
            attT = cpool.tile([128, 2 * HEADS], mybir.dt.float32)
            for h in range(HEADS):
                nc.sync.dma_start(out=attT[:, 2 * h:2 * h + 1], in_=att_src[h, :, None])
                nc.sync.dma_start(out=attT[:, 2 * h + 1:2 * h + 2], in_=att_dst[h, :, None])

            # WT quarters: WT[oh][:, f-slice]  (transpose W[fh, oh] blocks)
            for fh in range(2):
                for oh in range(2):
                    wtp = pp.tile([128, 128], mybir.dt.float32, name="wtp")
                    nc.tensor.transpose(out=wtp[:], in_=rhsW[fh][:, oh * 128:(oh + 1) * 128], identity=ident[:])
                    wt = pool.tile([128, 128], mybir.dt.float32, name="wt")
                    nc.vector.tensor_copy(out=wt[:], in_=wtp[:])
                    # Vsd columns for this (fh): head h = oh contract range
                    # haug col layout: 256+ : (s0, s1, d0, d1)
                    vsd = pp1.tile([128, 4], mybir.dt.float32, name="vsd", tag=f"vsd{fh}")
                    h = oh
                    nc.tensor.matmul(vsd[:, h:h + 1], lhsT=wt[:], rhs=attT[:, 2 * h:2 * h + 1],
                                     start=True, stop=True)
                    nc.tensor.matmul(vsd[:, 2 + h:2 + h + 1], lhsT=wt[:], rhs=attT[:, 2 * h + 1:2 * h + 2],
                                     start=True, stop=True)
                    nc.vector.tensor_copy(out=rhsW[fh][:, D + h:D + h + 1], in_=vsd[:, h:h + 1])
                    nc.vector.tensor_copy(out=rhsW[fh][:, D + 2 + h:D + 2 + h + 1], in_=vsd[:, 2 + h:2 + h + 1])

            for t in range(DTILES):
                xt = pool.tile([128, D], mybir.dt.float32, name="xt")
                nc.sync.dma_start(out=xt[:], in_=x[t * 128:(t + 1) * 128, :])
                hp = pp.tile([128, D + 4], mybir.dt.float32, name="hp")
                for fh in range(2):
                    xtp = pp.tile([128, 128], mybir.dt.float32, name="xtp")
                    nc.tensor.transpose(out=xtp[:], in_=xt[:, fh * 128:(fh + 1) * 128], identity=ident[:])
                    xts = pool.tile([128, 128], mybir.dt.float32, name="xts")
                    nc.vector.tensor_copy(out=xts[:], in_=xtp[:])
                    nc.tensor.matmul(hp[:], lhsT=xts[:], rhs=rhsW[fh][:],
                                     start=(fh == 0), stop=(fh == 1))
                ht = pool.tile([128, AUGW], DT, name="ht")
                nc.vector.memset(ht[:, D + 4:AUGW], 0.0)
                nc.vector.tensor_copy(out=ht[:, 0:D + 4], in_=hp[:])
                nc.sync.dma_start(out=haug[t * 128:(t + 1) * 128, :], in_=ht[:])
    nc.compile()
    return nc


# --------------------------------------------------------------------------
# Phase 2: edge aggregation + softmax + MLP
# --------------------------------------------------------------------------

def build_p2(W_T):
    S_T = DTILES * W_T * 128           # padded edge slots per core
    nc = _new_nc()
    f32 = mybir.dt.float32
    haug = nc.dram_tensor("haug", [N + 1, AUGW], DT, kind="ExternalInput")
    srcw = nc.dram_tensor("srcw", [128, S_T // 16], mybir.dt.int16, kind="ExternalInput")
    dstrel_cols = nc.dram_tensor("dstrel_cols", [S_T // 128, 128], f32, kind="ExternalInput")
    dstrel_flat = nc.dram_tensor("dstrel_flat", [1, S_T], DT, kind="ExternalInput")
    asd = nc.dram_tensor("asd", [NSH, 2], DT, kind="ExternalInput")
    b_rep_in = nc.dram_tensor("b_rep", [128, D], f32, kind="ExternalInput")
    Wa = nc.dram_tensor("Wa", [D, 128], f32, kind="ExternalInput")
    ba = nc.dram_tensor("ba", [128], f32, kind="ExternalInput")
    W1 = nc.dram_tensor("W1", [128, 64], f32, kind="ExternalInput")
    b1 = nc.dram_tensor("b1", [64], f32, kind="ExternalInput")
    W2 = nc.dram_tensor("W2", [64, 3], f32, kind="ExternalInput")
    b2 = nc.dram_tensor("b2", [3], f32, kind="ExternalInput")

    alpha_out = nc.dram_tensor("alpha", [NSH, W_T * 2], f32, kind="ExternalOutput")
    yT_out = nc.dram_tensor("yT", [3, NSH], f32, kind="ExternalOutput")

    # chunk layout per dst-tile: W_T edge-tiles split into chunks of <= CH
    chunks = []
    w0 = 0
    while w0 < W_T:
        chunks.append((w0, min(CH, W_T - w0)))
        w0 += chunks[-1][1]

    with tile.TileContext(nc) as tc:
        with (
            tc.tile_pool(name="const", bufs=1) as cpool,
            tc.tile_pool(name="sbuf", bufs=3) as pool,
            tc.tile_pool(name="keep", bufs=len(chunks) + 2) as kpool,
        ):
            ident = cpool.tile([128, 128], mybir.dt.float32)
            make_identity(nc, ident[:])
            iota_i = cpool.tile([128, 128], mybir.dt.int32)
            nc.gpsimd.iota(iota_i[:], pattern=[[1, 128]], base=0, channel_multiplier=0)
            iota_row = cpool.tile([128, 128], DT)
            nc.vector.tensor_copy(out=iota_row[:], in_=iota_i[:])
            iotac_i = cpool.tile([128, 1], mybir.dt.int32)
            nc.gpsimd.iota(iotac_i[:], pattern=[[1, 1]], base=0, channel_multiplier=1)
            iota_col = cpool.tile([128, 1], f32)
            nc.vector.tensor_copy(out=iota_col[:], in_=iotac_i[:])

            b_rep = cpool.tile([128, D], f32)
            nc.sync.dma_start(out=b_rep[:], in_=b_rep_in[:, :])
            h2T = [cpool.tile([128, NSH], f32, name=f"h2T{i}") for i in range(2)]

            agg_psum_ctx = (
                tc.tile_pool(name="psA", bufs=1, space="PSUM"),
                tc.tile_pool(name="psB", bufs=1, space="PSUM"),
            )
            psA, psB = [p.__enter__() for p in agg_psum_ctx]

            for t in range(DTILES):
                agg = psA.tile([128, D], f32, name="agg", tag="agg", bufs=2)
                den = psB.tile([128, 2], f32, name="den", tag="den", bufs=1)
                asd_t = pool.tile([128, 2], DT, name="asd_t")
                nc.sync.dma_start(out=asd_t[:], in_=asd[t * 128:(t + 1) * 128, :])
                wsc_tiles = []
                s2_tiles = []
                for ci, (w0, cw) in enumerate(chunks):
                    s0 = (t * W_T + w0) * 128
                    nidx = cw * 128
                    sw = pool.tile([128, nidx // 16], mybir.dt.int16, name="sw")
                    nc.sync.dma_start(out=sw[:], in_=srcw[:, s0 // 16:(s0 + nidx) // 16])
                    drel = pool.tile([128, cw], f32, name="drel")
                    nc.sync.dma_start(out=drel[:], in_=dstrel_cols[s0 // 128:s0 // 128 + cw, :].rearrange("w p -> p w"))
                    drep = pool.tile([128, CH * 128], DT, name="drep")
                    nc.sync.dma_start(out=drep[:, 0:nidx],
                                      in_=dstrel_flat[0:1, s0:s0 + nidx].to_broadcast([128, nidx]))

                    g1 = pool.tile([128, CH, AUGW], DT, name="g1", tag="g1")
                    nc.gpsimd.dma_gather(
                        out_ap=g1[:, 0:cw, :], in_ap=haug.ap(), idxs_ap=sw[:],
                        num_idxs=nidx, num_idxs_reg=nidx, elem_size=AUGW,
                    )
                    # S2[d, e] one-hot + a_d expansion via matmul
                    ade = psB.tile([128, 2 * CH], f32, name="ade", tag="ade", bufs=2)
                    cw_s2 = []
                    for w in range(cw):
                        S2 = kpool.tile([128, 128], DT, name="S2", tag="S2", bufs=W_T + 4)
                        nc.vector.tensor_scalar(S2[:], drep[:, w * 128:(w + 1) * 128],
                                                iota_col[:], None, ALU.is_equal)
                        nc.tensor.matmul(ade[:, 2 * w:2 * w + 2], lhsT=S2[:], rhs=asd_t[:],
                                         start=True, stop=True)
                        cw_s2.append(S2)
                    s2_tiles.append(cw_s2)
                    # e = leakyrelu(a_s[src] + a_d[dst]) = max(e0, 0.2*e0); w = exp(e)
                    esc = pool.tile([128, CH, 2], f32, name="esc")
                    nc.vector.tensor_tensor(out=esc[:, 0:cw, :], in0=g1[:, 0:cw, D:D + 2],
                                            in1=ade[:, 0:2 * cw].rearrange("p (w c) -> p w c", c=2),
                                            op=ALU.add)
                    esc2 = pool.tile([128, CH, 2], f32, name="esc2")
                    nc.vector.tensor_scalar(esc2[:, 0:cw, :], esc[:, 0:cw, :], 0.2, None, ALU.mult)
                    nc.vector.tensor_tensor(out=esc[:, 0:cw, :], in0=esc[:, 0:cw, :],
                                            in1=esc2[:, 0:cw, :], op=ALU.max)
                    wsc = kpool.tile([128, CH, 2], f32, name="wsc", tag="wsc")
                    nc.scalar.activation(wsc[:, 0:cw, :], esc[:, 0:cw, :], F.Exp)
                    wsc_tiles.append(wsc)
                    wbf = pool.tile([128, CH, 2], DT, name="wbf")
                    nc.vector.tensor_copy(out=wbf[:, 0:cw, :], in_=wsc[:, 0:cw, :])

                    for w in range(cw):
                        gw = w0 + w
                        S = pool.tile([128, 128], DT, name="S")
                        nc.vector.tensor_scalar(S[:], iota_row[:], drel[:, w:w + 1], None, ALU.is_equal)
                        rhsb = pool.tile([128, 2 * O], DT, name="rhsb")
                        nc.vector.tensor_scalar(rhsb[:, 0:O], g1[:, w, 0:O], wsc[:, w, 0:1], None, ALU.mult)
                        nc.vector.tensor_scalar(rhsb[:, O:2 * O], g1[:, w, O:2 * O], wsc[:, w, 1:2], None, ALU.mult)
                        first = gw == 0
                        last = gw == W_T - 1
                        nc.tensor.matmul(agg[:], lhsT=S[:], rhs=rhsb[:], start=first, stop=last)
                        nc.tensor.matmul(den[:], lhsT=S[:], rhs=wbf[:, w, 0:2], start=first, stop=last)

                # finalize dst-tile t
                dens = pool.tile([128, 2], f32, name="dens")
                nc.vector.tensor_scalar(dens[:], den[:], 1e-16, None, ALU.add)
                rde = pool.tile([128, 2], f32, name="rde")
                nc.vector.reciprocal(rde[:], dens[:])
                rde_dt = pool.tile([128, 2], DT, name="rde_dt")
                nc.vector.tensor_copy(out=rde_dt[:], in_=rde[:])

                h2t = pool.tile([128, D], f32, name="h2t")
                nc.vector.tensor_scalar(h2t[:, 0:O], agg[:, 0:O], rde[:, 0:1], None, ALU.mult)
                nc.vector.tensor_scalar(h2t[:, O:2 * O], agg[:, O:2 * O], rde[:, 1:2], None, ALU.mult)
                nc.vector.tensor_tensor(out=h2t[:], in0=h2t[:], in1=b_rep[:], op=ALU.add)
                nc.vector.tensor_scalar(h2t[:], h2t[:], 0.0, None, ALU.max)
                for fh in range(2):
                    htp = psA.tile([128, 128], f32, name="htp", tag="htp", bufs=1)
                    nc.tensor.transpose(out=htp[:], in_=h2t[:, fh * 128:(fh + 1) * 128], identity=ident[:])
                    nc.vector.tensor_copy(out=h2T[fh][:, t * 128:(t + 1) * 128], in_=htp[:])

                # alpha = w * rdenom[dst] (expand rdenom via kept S2 tiles)
                alpha_t = pool.tile([128, W_T, 2], f32, name="alpha_t")
                for ci, (w0, cw) in enumerate(chunks):
                    wsc = wsc_tiles[ci]
                    rdexp = psB.tile([128, 2 * CH], f32, name="rdexp", tag="rdexp", bufs=1)
                    for w in range(cw):
                        nc.tensor.matmul(rdexp[:, 2 * w:2 * w + 2], lhsT=s2_tiles[ci][w][:],
                                         rhs=rde_dt[:], start=True, stop=True)
                    nc.vector.tensor_tensor(out=alpha_t[:, w0:w0 + cw, :], in0=wsc[:, 0:cw, :],
                                            in1=rdexp[:, 0:2 * cw].rearrange("p (w c) -> p w c", c=2),
                                            op=ALU.mult)
                nc.sync.dma_start(
                    out=alpha_out[t * 128:(t + 1) * 128, :],
                    in_=alpha_t[:].rearrange("p w c -> p (w c)"),
                )

            for p in reversed(agg_psum_ctx):
                p.__exit__(None, None, None)

            # ---- MLP on h2T -> yT (DVE for bias+relu; no ACT tables) ----
            psM = tc.tile_pool(name="psM", bufs=2, space="PSUM")
            psM_pool = psM.__enter__()
            wa_t = [cpool.tile([128, 128], f32, name=f"wa{i}") for i in range(2)]
            for fh in range(2):
                nc.sync.dma_start(out=wa_t[fh][:], in_=Wa[fh * 128:(fh + 1) * 128, :])
            w1_t = cpool.tile([128, 64], f32)
            nc.sync.dma_start(out=w1_t[:], in_=W1[:, :])
            w2_t = cpool.tile([64, 3], f32)
            nc.sync.dma_start(out=w2_t[:], in_=W2[:, :])
            ba_c = cpool.tile([128, 1], f32)
            nc.sync.dma_start(out=ba_c[:], in_=ba[:, None])
            b1_c = cpool.tile([64, 1], f32)
            nc.sync.dma_start(out=b1_c[:], in_=b1[:, None])
            b2_c = cpool.tile([3, 1], f32)
            nc.sync.dma_start(out=b2_c[:], in_=b2[:, None])

            r1T = cpool.tile([128, NSH], f32)
            r2T = cpool.tile([64, NSH], f32)
            MLPC = min(512, NSH)
            for j in range(NSH // MLPC):
                sl = slice(j * MLPC, (j + 1) * MLPC)
                p1 = psM_pool.tile([128, MLPC], f32, name="p1", tag="mlp1")
                for fh in range(2):
                    nc.tensor.matmul(p1[:], lhsT=wa_t[fh][:], rhs=h2T[fh][:, sl],
                                     start=(fh == 0), stop=(fh == 1))
                nc.vector.tensor_scalar(r1T[:, sl], p1[:], ba_c[:], 0.0, ALU.add, ALU.max)
                p2 = psM_pool.tile([64, MLPC], f32, name="p2", tag="mlp2")
                nc.tensor.matmul(p2[:], lhsT=w1_t[:], rhs=r1T[:, sl], start=True, stop=True)
                nc.vector.tensor_scalar(r2T[:, sl], p2[:], b1_c[:], 0.0, ALU.add, ALU.max)
                p3 = psM_pool.tile([3, MLPC], f32, name="p3", tag="mlp3")
                nc.tensor.matmul(p3[:], lhsT=w2_t[:], rhs=r2T[:, sl], start=True, stop=True)
                yt = pool.tile([3, MLPC], f32, name="yt")
                nc.vector.tensor_scalar(yt[:], p3[:], b2_c[:], None, ALU.add)
                nc.sync.dma_start(out=yT_out[:, sl], in_=yt[:])
            psM.__exit__(None, None, None)
    nc.compile()
    return nc


# --------------------------------------------------------------------------
# Phase 3: dist block = cdist(y_own, y_all)
# --------------------------------------------------------------------------

def build_p3():
    nc = _new_nc()
    f32 = mybir.dt.float32
    yT = nc.dram_tensor("yT", [3, N], f32, kind="ExternalInput")
    yTo = nc.dram_tensor("yTo", [3, NSH], f32, kind="ExternalInput")  # own block
    dist = nc.dram_tensor("dist", [NSH, N], f32, kind="ExternalOutput")

    with tile.TileContext(nc) as tc:
        with (
            tc.tile_pool(name="const", bufs=1) as cpool,
            tc.tile_pool(name="sbuf", bufs=4) as pool,
            tc.tile_pool(name="psum", bufs=4, space="PSUM") as pp,
        ):
            # rhs_cd rows: y (3), ones, y2   [5, N]
            # compute-engine APs must start at partition 0; assemble rows 3,4
            # via SBUF->SBUF DMA from base-0 tiles.
            rhs_cd = cpool.tile([5, N], f32)
            nc.sync.dma_start(out=rhs_cd[0:3, :], in_=yT[:, :])
            CKW = 2048
            ones_c = cpool.tile([1, CKW], f32)
            nc.vector.memset(ones_c[:], 1.0)

            def sumsq_row(src3, dst_slice):  # src3/dst [.., CKW]
                sq = pool.tile([3, CKW], f32, name="sq", tag="sq", bufs=2)
                nc.vector.tensor_tensor(out=sq[:], in0=src3, in1=src3, op=ALU.mult)
                t1 = pool.tile([1, CKW], f32, name="t1", tag="t1", bufs=2)
                nc.sync.dma_start(out=t1[:], in_=sq[1:2, :])
                t2 = pool.tile([1, CKW], f32, name="t2", tag="t2", bufs=2)
                nc.sync.dma_start(out=t2[:], in_=sq[2:3, :])
                y2r = pool.tile([1, CKW], f32, name="y2r", tag="y2r", bufs=2)
                nc.vector.tensor_tensor(out=y2r[:], in0=sq[0:1, :], in1=t1[:], op=ALU.add)
                nc.vector.tensor_tensor(out=y2r[:], in0=y2r[:], in1=t2[:], op=ALU.add)
                nc.sync.dma_start(out=dst_slice, in_=y2r[:])

            for ck in range(N // CKW):
                cs = slice(ck * CKW, (ck + 1) * CKW)
                nc.sync.dma_start(out=rhs_cd[3:4, cs], in_=ones_c[:])
                sumsq_row(rhs_cd[0:3, cs], rhs_cd[4:5, cs])

            # lhsT_cd rows: -2y_own (3), y2_own, ones  [5, NSH]
            lhs_cd = cpool.tile([5, NSH], f32)
            yto_sb = cpool.tile([3, NSH], f32)
            nc.sync.dma_start(out=yto_sb[:], in_=yTo[:, :])
            for ck in range(NSH // CKW):
                cs = slice(ck * CKW, (ck + 1) * CKW)
                nc.sync.dma_start(out=lhs_cd[4:5, cs], in_=ones_c[:])
                sumsq_row(yto_sb[:, cs], lhs_cd[3:4, cs])
            nc.vector.tensor_scalar(lhs_cd[0:3, :], yto_sb[:], -2.0, None, ALU.mult)

            for i in range(DTILES):
                for j in range(N // 512):
                    d2 = pp.tile([128, 512], f32, name="d2")
                    nc.tensor.matmul(d2[:], lhsT=lhs_cd[:, i * 128:(i + 1) * 128],
                                     rhs=rhs_cd[:, j * 512:(j + 1) * 512], start=True, stop=True)
                    dcl = pool.tile([128, 512], f32, name="dcl")
                    nc.vector.tensor_scalar(dcl[:], d2[:], 0.0, None, ALU.max)
                    dst_t = pool.tile([128, 512], f32, name="dst_t")
                    nc.scalar.sqrt(dst_t[:], dcl[:])
                    nc.sync.dma_start(out=dist[i * 128:(i + 1) * 128, j * 512:(j + 1) * 512], in_=dst_t[:])
    nc.compile()
    return nc


# --------------------------------------------------------------------------
# Host orchestration
# --------------------------------------------------------------------------

def _get(phase, key, builder):
    k = (phase, key)
    if k not in _compiled:
        _compiled[k] = builder()
    return _compiled[k]


def _run(nc, in_maps):
    global LAST_EXEC_NS
    if TRACE:
        _install_ntff_hook()
    res = run_bass_kernel_spmd(nc, in_maps, core_ids=list(range(NCORES)), trace=TRACE)
    if TRACE:
        LAST_EXEC_NS.append(res.exec_time_ns)
    return res.results


def host_prep(edge_index):
    """Self loops + sort by dst + pad per dst-tile to uniform W_T*128 slots."""
    ar = np.arange(N, dtype=np.int64)
    ei = np.concatenate([np.asarray(edge_index, np.int64), np.stack([ar, ar])], axis=1)
    Ep = ei.shape[1]
    src_a = ei[0].astype(np.int64)
    dst_a = ei[1].astype(np.int64)
    order = np.argsort(dst_a, kind="stable")
    src_s = src_a[order].astype(np.int32)
    dst_s = dst_a[order].astype(np.int32)
    gtile = dst_s >> 7                                  # global dst-tile
    ntiles = N // 128
    counts = np.bincount(gtile, minlength=ntiles)
    W_T = int(np.ceil(counts.max() / 128))
    S_T = DTILES * W_T * 128
    starts = np.zeros(ntiles, np.int64)
    starts[1:] = np.cumsum(counts)[:-1]
    slot = gtile.astype(np.int64) * (W_T * 128) + (np.arange(Ep) - starts[gtile])

    S_ALL = ntiles * W_T * 128
    src_slots = np.full(S_ALL, N, np.int32)
    src_slots[slot] = src_s
    dstrel_slots = np.zeros(S_ALL, np.int32)
    dstrel_slots[slot] = dst_s & 127
    tile_of_slot = np.arange(S_ALL, dtype=np.int64) // (W_T * 128)
    dstloc_slots = ((tile_of_slot % DTILES) * 128).astype(np.int32)
    dstloc_slots[slot] = dst_s & (NSH - 1)
    return dict(ei=ei, order=order, slot=slot, W_T=W_T, S_T=S_T, S_ALL=S_ALL,
                src_slots=src_slots, dstrel_slots=dstrel_slots,
                dstloc_slots=dstloc_slots)


def wrap16(a):  # [S_T] -> [128, S_T//16] int16
    w = a.reshape(-1, 16).T.astype(np.int16)            # [16, S_T//16]
    return np.tile(w, (8, 1)).copy()


def kernel(x, edge_index, W, b, att_src, att_dst, Wa, ba, W1, b1, W2, b2):
    global LAST_EXEC_NS
    LAST_EXEC_NS = []
    npdt = _np_of(DT)
    x = np.asarray(x, np.float32)
    W = np.asarray(W, np.float32)
    b = np.asarray(b, np.float32)
    att_src = np.asarray(att_src, np.float32)
    att_dst = np.asarray(att_dst, np.float32)
    Wa = np.asarray(Wa, np.float32); ba = np.asarray(ba, np.float32)
    W1 = np.asarray(W1, np.float32); b1 = np.asarray(b1, np.float32)
    W2 = np.asarray(W2, np.float32); b2 = np.asarray(b2, np.float32)
    ei_dt = np.asarray(edge_index).dtype

    hp = host_prep(edge_index)
    ei = hp["ei"]; order = hp["order"]; slot = hp["slot"]
    W_T = hp["W_T"]; S_T = hp["S_T"]; S_ALL = hp["S_ALL"]
    src_slots = hp["src_slots"]; dstrel_slots = hp["dstrel_slots"]
    dstloc_slots = hp["dstloc_slots"]
    Ep = ei.shape[1]

    # ---- phase 1 ----
    nc1 = _get("p1", DT, build_p1)
    p1_in = [
        {"x": x[c * NSH:(c + 1) * NSH], "W": W, "att_src": att_src, "att_dst": att_dst}
        for c in range(NCORES)
    ]
    r1 = _run(nc1, p1_in)
    haug_sh = [r1[c]["haug"] for c in range(NCORES)]
    pad_row = np.zeros((1, AUGW), npdt)
    pad_row[0, D:D + 2] = -100.0                   # a_s sentinel -> w=exp(-20)~0
    haug_full = np.concatenate(haug_sh + [pad_row], axis=0)

    # ---- phase 2 ----
    nc2 = _get("p2", (DT, W_T), lambda: build_p2(W_T))
    b_rep = np.tile(b[None, :], (128, 1)).astype(np.float32)
    p2_in = []
    for c in range(NCORES):
        sl = slice(c * S_T, (c + 1) * S_T)
        drel = dstrel_slots[sl]
        p2_in.append({
            "haug": haug_full,
            "srcw": wrap16(src_slots[sl]),
            "dstrel_cols": drel.reshape(-1, 128).astype(np.float32),
            "dstrel_flat": drel.reshape(1, -1).astype(npdt),
            "asd": np.ascontiguousarray(haug_full[c * NSH:(c + 1) * NSH, D + 2:D + 4]),
            "b_rep": b_rep, "Wa": Wa, "ba": ba, "W1": W1, "b1": b1, "W2": W2, "b2": b2,
        })
    r2 = _run(nc2, p2_in)
    yT_full = np.concatenate([r2[c]["yT"] for c in range(NCORES)], axis=1)  # [3, N]

    # alpha: [NSH, W_T*2] per core -> per-slot -> unsort
    alpha_slots = np.empty((S_ALL, 2), np.float32)
    for c in range(NCORES):
        a = r2[c]["alpha"].reshape(NSH, W_T, 2)          # [p-major within tile]
        a = a.reshape(DTILES, 128, W_T, 2).transpose(0, 2, 1, 3).reshape(S_T, 2)
        alpha_slots[c * S_T:(c + 1) * S_T] = a
    alpha_sorted = alpha_slots[slot]                     # [Ep, 2] in sorted order
    alpha = np.empty((Ep, HEADS), np.float32)
    alpha[order] = alpha_sorted

    # ---- phase 3 ----
    nc3 = _get("p3", DT, build_p3)
    p3_in = [{"yT": yT_full, "yTo": np.ascontiguousarray(yT_full[:, c * NSH:(c + 1) * NSH])}
             for c in range(NCORES)]
    r3 = _run(nc3, p3_in)
    dist = np.concatenate([r3[c]["dist"] for c in range(NCORES)], axis=0)

    return dist, (ei.astype(ei_dt), alpha)
